# revision 31
# baseline (speedup 1.0000x reference)
"""Trainium2 Bass kernel for nn_DiverseRegDCConv2d.

Per-sample dynamic 3x3 conv: filters are generated per sample from an
8-column weight bank (wgen[b] = se[b] @ bank.T), then applied as a
standard 256->256 conv on 28x28 with padding 1.

Sharding (8 cores): 4 batch-groups x 2 out-channel halves. Each core
handles 8 samples x 128 out channels; the weight bank half it needs is
replicated across the 4 batch-groups. No cross-device communication.

Numerics: the conv runs on fp8e4 (e4m3) operands in DoubleRow perf
mode (K=256 per matmul, 0.5 cycles per output column) with a 3-term
residual split that recovers ~fp16 accuracy:

    out = (w8 (.) x8  +  w8 (.) dx8  +  dwq (.) x8) / 16

where the filters are generated on device at 16x scale (se pre-scaled
on host), w8 = fp8(W), dwq = fp8(W - w8), and the activations are
split on host as x8 = fp8(x), dx8 = fp8(x - x8). End-to-end relative
error vs the fp32 reference is ~1.5e-3.

Conv layout trick: activations are stored width-29 row-flattened
(left zero-pad column only; the right pad of row r aliases the left
zero of row r+1), so every 3x3 shifted window is a single contiguous
406-element run and the DoubleRow moving AP stays 3-D. Each PSUM row
has one discarded halo column (29 vs 28).

Filter generation stays on device and is SAMPLE-HALF blocked: the
block-diagonal 16*se operand's columns are sample-major, so a 64-col
slice generates filters for samples 0-3 (then 4-7) across all k.
Samples 2-3's conv bursts therefore overlap the second wgen half, and
samples 4-7's bursts run immediately after it — the PE never waits
long on filter generation. Evacuation is three passes so PSUM turns
around fast and the fp8 ops run SBUF->SBUF: (1) Act copy PSUM->W16,
(2) DVE quantize W16->w8 (TensorCopy in 2x all-SBUF mode),
(3) DVE/Pool subtract W16-w8 -> dwq.
"""

import sys

for _p in ("/opt/trn_rl_repo", "/root/.axon_site/_ro/trn_rl_repo"):
    if _p not in sys.path:
        sys.path.append(_p)

import ml_dtypes
import numpy as np

import concourse.bass as bass
import concourse.mybir as mybir
from concourse import bacc
from concourse.bass_utils import run_bass_kernel_spmd
from concourse.tile import TileContext

B, C, O, KS, H, W, NUM = 32, 256, 256, 3, 28, 28, 8
P = 128
NCORES = 8
BG, OHALF = 4, 2          # batch-groups x out-channel halves
S = B // BG               # samples per core = 8
OC = O // OHALF           # out channels per core = 128
CC = C // P               # input-channel chunks = 2
G = 16                    # (k,o)-blocks per wgen matmul column group
NK = KS * KS              # 9 kernel positions
SH = S // 2               # samples per wgen half = 4

FW = W + 1                # flat row width (left zero col, right pad aliased)
NR = H + 2                # padded rows
FLAT = NR * FW + 2        # + guard zeros for the bottom-right window overrun
HO = H // 2               # 14 output rows per psum group
NF = HO * FW              # 406 psum columns per group (1 halo col per row)
HFLAT = (HO + 2) * FW + 2  # one output-half's input rows (16) + guards

NPROG = 2                 # samples whose conv groups run k-progressively

F32 = mybir.dt.float32
F16 = mybir.dt.float16
F8 = mybir.dt.float8e4
E4NP = ml_dtypes.float8_e4m3

_NC = None


def _build_nc():
    nc = bacc.Bacc()
    xq_d = nc.declare_dram_parameter("xq", [S, P, 2, CC, FLAT], F8, isOutput=False)
    wp_d = nc.declare_dram_parameter("wp", [NK, P, CC, 8, P], F16, isOutput=False)
    se_d = nc.declare_dram_parameter("sebd", [P, P], F16, isOutput=False)
    b_d = nc.declare_dram_parameter("bias", [P, 1], F32, isOutput=False)
    out_d = nc.declare_dram_parameter("out", [S, P, H, W], F16, isOutput=True)

    with TileContext(nc) as tc:
        with (
            tc.tile_pool(name="constp", bufs=1) as constp,
            tc.tile_pool(name="wstream", bufs=9) as wstream,
            tc.tile_pool(name="xpool", bufs=1) as xpool,
            tc.tile_pool(name="slabp", bufs=1) as slabp,
            tc.tile_pool(name="outp", bufs=4) as outp,
            tc.tile_pool(name="wgps", bufs=2, space="PSUM") as wgps,
            tc.tile_pool(name="cvps", bufs=1, space="PSUM") as cvps,
        ):
            # slabs: [c_lo, cc, k, s, oc]; conv lhsT = slab[:, :, k, s, :]
            w16 = slabp.tile([P, CC, NK, S, OC], F16)
            w8 = slabp.tile([P, CC, NK, S, OC], F8)
            dwq = slabp.tile([P, CC, NK, S, OC], F8)

            xts = {}   # (s, hi) -> (x8 view, dx8 view, half_layout)

            def emit_xload(s):
                # x8 and dx8 ride one DMA (fewer HWDGE setups); the conv
                # slices views of the combined tile
                xq = xpool.tile([P, 2, CC, FLAT], F8, name=f"xq_{s}",
                                tag=f"xq_{s}")
                nc.sync.dma_start(out=xq, in_=xq_d[s, :, :, :, :])
                for hi in range(2):
                    xts[(s, hi)] = (xq[:, 0], xq[:, 1], False)

            def emit_xload_half(s, hi):
                # one output-half's input rows only: finer DMA granularity so
                # early conv groups start sooner on the saturated front bus
                xq = xpool.tile([P, 2, CC, HFLAT], F8, name=f"xq_{s}_{hi}",
                                tag=f"xq_{s}_{hi}")
                off = hi * HO * FW
                nc.sync.dma_start(out=xq, in_=xq_d[s, :, :, :, off:off + HFLAT])
                xts[(s, hi)] = (xq[:, 0], xq[:, 1], True)

            def emit_wload(k):
                wt = wstream.tile([P, CC, 8, P], F16, name=f"wp_{k}", tag="wp")
                nc.sync.dma_start(out=wt, in_=wp_d[k, :, :, :, :])
                return wt

            # wp k=0 cc=0 leads the DMA queue so filter generation starts
            # ASAP; se/bias slot into its shadow, then the progressive
            # samples' activations stream behind wp k=1.
            wts = {}
            wt0 = wstream.tile([P, CC, 8, P], F16, name="wp_0", tag="wp")
            nc.sync.dma_start(out=wt0[:, 0], in_=wp_d[0, :, 0, :, :])
            se_sb = constp.tile([P, P], F16)
            nc.sync.dma_start(out=se_sb, in_=se_d[:, :])
            bias_sb = constp.tile([P, 1], F32)
            nc.sync.dma_start(out=bias_sb, in_=b_d[:, :])
            nc.sync.dma_start(out=wt0[:, 1], in_=wp_d[0, :, 1, :, :])
            wts[0] = wt0
            wts[1] = emit_wload(1)
            emit_xload_half(0, 0)
            wts[2] = emit_wload(2)
            emit_xload_half(1, 0)
            emit_xload_half(0, 1)
            wts[3] = emit_wload(3)
            emit_xload_half(1, 1)
            wts[4] = emit_wload(4)
            emit_xload_half(2, 0)
            wts[5] = emit_wload(5)
            emit_xload_half(2, 1)
            wts[6] = emit_wload(6)
            wts[7] = emit_wload(7)
            wts[8] = emit_wload(8)
            emit_xload_half(3, 0)
            emit_xload_half(3, 1)

            def emit_wgen(cc, k, sh, wt):
                # one sample-half: 8 matmuls of 64 cols -> 1 psum tile
                ps = wgps.tile([P, 8 * (SH * G)], F32)
                sse = se_sb[:, sh * SH * G:(sh + 1) * SH * G]
                for i in range(8):
                    nc.tensor.matmul(
                        ps[:, i * 64:(i + 1) * 64], wt[:, cc, i, :],
                        sse, start=True, stop=True,
                    )
                # pass 1 (Act): PSUM -> fp16 W slab, frees PSUM fast
                src = ps.rearrange("p (i s g) -> p i s g", i=8, s=SH, g=G)
                dst = w16[:, cc, k, sh * SH:(sh + 1) * SH, :].rearrange(
                    "p s (i g) -> p i s g", g=G)
                nc.scalar.activation(
                    dst, src, mybir.ActivationFunctionType.Identity)

            def emit_pass23(cc, k, sh):
                # pass 2 (DVE): w8 = fp8(W16) -- TensorCopy 2x all-SBUF mode.
                # pass 3: dwq = fp8(W16 - w8): DVE for cc0, Pool for cc1.
                wsrc = w16[:, cc, k, sh * SH:(sh + 1) * SH].rearrange(
                    "p s o -> p (s o)")
                wdst = w8[:, cc, k, sh * SH:(sh + 1) * SH].rearrange(
                    "p s o -> p (s o)")
                ddst = dwq[:, cc, k, sh * SH:(sh + 1) * SH].rearrange(
                    "p s o -> p (s o)")
                nc.vector.tensor_copy(out=wdst, in_=wsrc)
                eng = nc.vector if cc == 0 else nc.gpsimd
                eng.tensor_tensor(ddst, wsrc, wdst, mybir.AluOpType.subtract)

            def emit_conv_term(k, s, hi, pst, ti, first=False, last=False):
                ky, kx = k // KS, k % KS
                xv, dxv, half = xts[(s, hi)]
                st = (ky if half else hi * HO + ky) * FW + kx
                stat, mov = ((w8, xv), (w8, dxv), (dwq, xv))[ti]
                nc.tensor.matmul(
                    pst, stat[:, :, k, s, :], mov[:, :, st:st + NF],
                    start=first, stop=last,
                    perf_mode=mybir.MatmulPerfMode.DoubleRow,
                    skip_group_check=True,
                )

            outts = [None] * S

            def emit_evac(s, hi, pst):
                if hi == 0:
                    outts[s] = outp.tile([P, 2, HO, W], F16, name=f"ot_{s}",
                                         tag="ot")
                nc.scalar.activation(
                    outts[s][:, hi],
                    pst.rearrange("p (h w) -> p h w", w=FW)[:, :, 0:W],
                    mybir.ActivationFunctionType.Identity,
                    bias=bias_sb[:, 0:1], scale=1.0 / 16.0,
                )
                # per-half store so the final group's DMA tail is short
                nc.sync.dma_start(
                    out=out_d[s, :, hi * HO:(hi + 1) * HO, :],
                    in_=outts[s][:, hi],
                )

            _ctag = [0]

            def new_group():
                t = cvps.tile([P, NF], F32, name=f"cv{_ctag[0]}",
                              tag=f"cv_{_ctag[0] % 6}")
                _ctag[0] += 1
                return t

            # ---- phase A: wgen half 0 (samples 0-3) + progressive conv of
            # six staggered groups whose starts track their x-half DMAs.
            # LAGS[(s, hi)] = k-lag of the main/dx terms (dw is one more).
            LAGS = {(0, 0): 1, (1, 0): 2, (0, 1): 3,
                    (1, 1): 4, (2, 0): 5, (2, 1): 6}
            prog = {g: new_group() for g in LAGS}
            for k in range(NK):
                convs = []
                for (s, hi), lag in LAGS.items():
                    if k >= lag:
                        convs.append((k - lag, s, hi, 0, k == lag))
                        convs.append((k - lag, s, hi, 1, False))
                    if k >= lag + 1:
                        convs.append((k - lag - 1, s, hi, 2, False))

                def chunk(n):
                    for _ in range(n):
                        if convs:
                            ck, cs, chi, cti, cf = convs.pop(0)
                            emit_conv_term(ck, cs, chi, prog[(cs, chi)], cti,
                                           first=cf)

                emit_wgen(0, k, 0, wts[k])
                chunk(6)
                emit_wgen(1, k, 0, wts[k])
                emit_pass23(0, k, 0)
                emit_pass23(1, k, 0)
                if k < 4:
                    emit_xload(4 + k)
                chunk(len(convs))
            for (s, hi) in ((0, 0), (1, 0)):
                lag = LAGS[(s, hi)]
                pst = prog[(s, hi)]
                for kk in range(NK - lag, NK):
                    emit_conv_term(kk, s, hi, pst, 0)
                    emit_conv_term(kk, s, hi, pst, 1)
                for kk in range(NK - lag - 1, NK):
                    emit_conv_term(kk, s, hi, pst, 2,
                                   last=(kk == NK - 1))
                emit_evac(s, hi, pst)

            # ---- phase B: wgen half 1 (samples 4-7) interleaved with the
            # rest of the staggered groups plus the bursts of s3 (slab half
            # 0 is complete). Groups carry (tile, op-list, evac target).
            bqueue = []
            for (s, hi) in ((0, 1), (1, 1), (2, 0), (2, 1)):
                lag = LAGS[(s, hi)]
                ops = ([(kk, s, hi, ti) for kk in range(NK - lag, NK)
                        for ti in (0, 1)]
                       + [(kk, s, hi, 2) for kk in range(NK - lag - 1, NK)])
                bqueue.append((prog[(s, hi)], ops, s, hi))
            for hi in range(2):
                bqueue.append((None, [(k, 3, hi, ti) for k in range(NK)
                                      for ti in range(3)], 3, hi))

            bstate = {"cur": None, "ops": None, "s": 0, "hi": 0}

            def bchunk(n):
                while n > 0:
                    if bstate["cur"] is None:
                        if not bqueue:
                            return
                        tile, ops, s, hi = bqueue.pop(0)
                        bstate["cur"] = tile if tile is not None else new_group()
                        bstate["ops"] = list(ops)
                        bstate["s"], bstate["hi"] = s, hi
                        if tile is None:
                            ck, cs, chi, cti = bstate["ops"].pop(0)
                            emit_conv_term(ck, cs, chi, bstate["cur"], cti,
                                           first=True)
                    while n > 0 and bstate["ops"]:
                        ck, cs, chi, cti = bstate["ops"].pop(0)
                        last = not bstate["ops"]
                        emit_conv_term(ck, cs, chi, bstate["cur"], cti,
                                       last=last)
                        n -= 1
                    if not bstate["ops"]:
                        emit_evac(bstate["s"], bstate["hi"], bstate["cur"])
                        bstate["cur"] = None

            for k in range(NK):
                emit_wgen(0, k, 1, wts[k])
                bchunk(6)
                emit_wgen(1, k, 1, wts[k])
                emit_pass23(0, k, 1)
                emit_pass23(1, k, 1)
                bchunk(6)
            bchunk(10 ** 6)

            # ---- phase C: conv bursts of samples 4-7.
            for s in range(SH, S):
                for hi in range(2):
                    pst = new_group()
                    for k in range(NK):
                        for ti in range(3):
                            emit_conv_term(k, s, hi, pst, ti,
                                           first=(k == 0 and ti == 0),
                                           last=(k == NK - 1 and ti == 2))
                    if (s, hi) == (S - 1, 1):
                        # split the very last evacuation so the exposed
                        # evac+DMA tail after the final matmul is shorter
                        pv = pst.rearrange("p (h w) -> p h w", w=FW)
                        for r0, r1 in ((0, 7), (7, HO)):
                            nc.scalar.activation(
                                outts[s][:, 1, r0:r1],
                                pv[:, r0:r1, 0:W],
                                mybir.ActivationFunctionType.Identity,
                                bias=bias_sb[:, 0:1], scale=1.0 / 16.0,
                            )
                            nc.sync.dma_start(
                                out=out_d[s, :, HO + r0:HO + r1, :],
                                in_=outts[s][:, 1, r0:r1],
                            )
                    else:
                        emit_evac(s, hi, pst)

    nc.compile()
    return nc


def _get_nc():
    global _NC
    if _NC is None:
        _NC = _build_nc()
    return _NC


def _prep_core_inputs(inputs, inputs_se, weight, bias, bg, oh):
    # weight rows: r = o*(C*9) + c*9 + (ky*3+kx)  -> [O, C, 3, 3, NUM]
    wr = weight.reshape(O, C, KS, KS, NUM)
    wo = wr[oh * OC:(oh + 1) * OC].reshape(OC, C, NK, NUM)  # [o, c, k, n]
    # [j, g, cc, c_lo, k, n] -> [k, n, g, cc, j, c_lo]; p = n*16+g
    t = wo.reshape(8, G, CC, P, NK, NUM)
    wp = t.transpose(4, 5, 1, 2, 0, 3).reshape(NK, P, CC, 8, P)
    wp = np.ascontiguousarray(wp.astype(np.float16))

    # block-diagonal 16*se: [(n,g), (s,g')] nonzero iff g==g'
    se16 = (16.0 * inputs_se[bg * S:(bg + 1) * S]).astype(np.float32)  # [s, n]
    sebd = np.zeros((NUM, G, S, G), dtype=np.float32)
    for g in range(G):
        sebd[:, g, :, g] = se16.T
    sebd = sebd.reshape(P, P).astype(np.float16)

    # activations: fp8 split, width-29 row-flat layout with guard zeros
    x_core = inputs[bg * S:(bg + 1) * S].astype(np.float32)
    x8 = x_core.astype(E4NP)
    dx8 = (x_core - x8.astype(np.float32)).astype(E4NP)

    def to_flat(a):
        f = np.zeros((S, CC, P, NR, FW), dtype=E4NP)
        f[:, :, :, 1:H + 1, 1:W + 1] = a.reshape(S, CC, P, H, W)
        out = np.zeros((S, CC, P, FLAT), dtype=E4NP)
        out[:, :, :, :NR * FW] = f.reshape(S, CC, P, NR * FW)
        return out.transpose(0, 2, 1, 3)

    xq = np.ascontiguousarray(
        np.stack([to_flat(x8), to_flat(dx8)], axis=2))  # [S, P, 2, CC, FLAT]

    return {
        "xq": xq,
        "wp": wp,
        "sebd": sebd,
        "bias": np.ascontiguousarray(
            bias[oh * OC:(oh + 1) * OC].reshape(OC, 1), dtype=np.float32
        ),
    }


def kernel(inputs, inputs_se, weight, bias):
    inputs = np.asarray(inputs, dtype=np.float32)
    inputs_se = np.asarray(inputs_se, dtype=np.float32)
    weight = np.asarray(weight, dtype=np.float32)
    bias = np.asarray(bias, dtype=np.float32)

    nc = _get_nc()
    in_maps = []
    for core in range(NCORES):
        bg, oh = core // OHALF, core % OHALF
        in_maps.append(_prep_core_inputs(inputs, inputs_se, weight, bias, bg, oh))

    res = run_bass_kernel_spmd(nc, in_maps, list(range(NCORES))).results

    out = np.empty((B, O, H, W), dtype=np.float32)
    for core in range(NCORES):
        bg, oh = core // OHALF, core % OHALF
        out[bg * S:(bg + 1) * S, oh * OC:(oh + 1) * OC] = (
            res[core]["out"].astype(np.float32))
    return out


# revision 34
# speedup vs baseline: 1.0105x; 1.0105x over previous
"""Trainium2 Bass kernel for nn_DiverseRegDCConv2d.

Per-sample dynamic 3x3 conv: filters are generated per sample from an
8-column weight bank (wgen[b] = se[b] @ bank.T), then applied as a
standard 256->256 conv on 28x28 with padding 1.

Sharding (8 cores): 4 batch-groups x 2 out-channel halves. Each core
handles 8 samples x 128 out channels; the weight bank half it needs is
replicated across the 4 batch-groups. No cross-device communication.

Numerics: the conv runs on fp8e4 (e4m3) operands in DoubleRow perf
mode (K=256 per matmul, 0.5 cycles per output column) with a 3-term
residual split that recovers ~fp16 accuracy:

    out = (w8 (.) x8  +  w8 (.) dx8  +  dwq (.) x8) / 16

where the filters are generated on device at 16x scale (se pre-scaled
on host), w8 = fp8(W), dwq = fp8(W - w8), and the activations are
split on host as x8 = fp8(x), dx8 = fp8(x - x8). End-to-end relative
error vs the fp32 reference is ~1.5e-3.

Conv layout trick: activations are stored width-29 row-flattened
(left zero-pad column only; the right pad of row r aliases the left
zero of row r+1), so every 3x3 shifted window is a single contiguous
406-element run and the DoubleRow moving AP stays 3-D. Each PSUM row
has one discarded halo column (29 vs 28).

Filter generation stays on device and is SAMPLE-HALF blocked: the
block-diagonal 16*se operand's columns are sample-major, so a 64-col
slice generates filters for samples 0-3 (then 4-7) across all k.
Samples 2-3's conv bursts therefore overlap the second wgen half, and
samples 4-7's bursts run immediately after it — the PE never waits
long on filter generation. Evacuation is three passes so PSUM turns
around fast and the fp8 ops run SBUF->SBUF: (1) Act copy PSUM->W16,
(2) DVE quantize W16->w8 (TensorCopy in 2x all-SBUF mode),
(3) DVE/Pool subtract W16-w8 -> dwq.
"""

import sys

for _p in ("/opt/trn_rl_repo", "/root/.axon_site/_ro/trn_rl_repo"):
    if _p not in sys.path:
        sys.path.append(_p)

import ml_dtypes
import numpy as np

import concourse.bass as bass
import concourse.mybir as mybir
from concourse import bacc
from concourse.bass_utils import run_bass_kernel_spmd
from concourse.tile import TileContext

B, C, O, KS, H, W, NUM = 32, 256, 256, 3, 28, 28, 8
P = 128
NCORES = 8
BG, OHALF = 4, 2          # batch-groups x out-channel halves
S = B // BG               # samples per core = 8
OC = O // OHALF           # out channels per core = 128
CC = C // P               # input-channel chunks = 2
G = 16                    # (k,o)-blocks per wgen matmul column group
NK = KS * KS              # 9 kernel positions
SH = S // 2               # samples per wgen half = 4

FW = W + 1                # flat row width (left zero col, right pad aliased)
NR = H + 2                # padded rows
FLAT = NR * FW + 2        # + guard zeros for the bottom-right window overrun
HO = H // 2               # 14 output rows per psum group
NF = HO * FW              # 406 psum columns per group (1 halo col per row)
HFLAT = (HO + 2) * FW + 2  # one output-half's input rows (16) + guards

NPROG = 2                 # samples whose conv groups run k-progressively

F32 = mybir.dt.float32
F16 = mybir.dt.float16
F8 = mybir.dt.float8e4
E4NP = ml_dtypes.float8_e4m3

_NC = None


def _build_nc():
    nc = bacc.Bacc()
    xq_d = nc.declare_dram_parameter("xq", [S, P, 2, CC, FLAT], F8, isOutput=False)
    wp_d = nc.declare_dram_parameter("wp", [NK, P, CC, 8, P], F16, isOutput=False)
    se_d = nc.declare_dram_parameter("sebd", [P, P], F16, isOutput=False)
    b_d = nc.declare_dram_parameter("bias", [P, 1], F32, isOutput=False)
    out_d = nc.declare_dram_parameter("out", [S, P, H, W], F16, isOutput=True)

    with TileContext(nc) as tc:
        with (
            tc.tile_pool(name="constp", bufs=1) as constp,
            tc.tile_pool(name="wstream", bufs=9) as wstream,
            tc.tile_pool(name="xpool", bufs=1) as xpool,
            tc.tile_pool(name="slabp", bufs=1) as slabp,
            tc.tile_pool(name="outp", bufs=4) as outp,
            tc.tile_pool(name="wgps", bufs=2, space="PSUM") as wgps,
            tc.tile_pool(name="cvps", bufs=1, space="PSUM") as cvps,
        ):
            # slabs: [c_lo, cc, k, s, oc]; conv lhsT = slab[:, :, k, s, :]
            w16 = slabp.tile([P, CC, NK, S, OC], F16)
            w8 = slabp.tile([P, CC, NK, S, OC], F8)
            dwq = slabp.tile([P, CC, NK, S, OC], F8)

            xts = {}   # (s, hi) -> (x8 view, dx8 view, half_layout)

            def emit_xload(s):
                # x8 and dx8 ride one DMA (fewer HWDGE setups); the conv
                # slices views of the combined tile
                xq = xpool.tile([P, 2, CC, FLAT], F8, name=f"xq_{s}",
                                tag=f"xq_{s}")
                nc.sync.dma_start(out=xq, in_=xq_d[s, :, :, :, :])
                for hi in range(2):
                    xts[(s, hi)] = (xq[:, 0], xq[:, 1], False)

            def emit_xload_half(s, hi):
                # one output-half's input rows only: finer DMA granularity so
                # early conv groups start sooner on the saturated front bus
                xq = xpool.tile([P, 2, CC, HFLAT], F8, name=f"xq_{s}_{hi}",
                                tag=f"xq_{s}_{hi}")
                off = hi * HO * FW
                nc.sync.dma_start(out=xq, in_=xq_d[s, :, :, :, off:off + HFLAT])
                xts[(s, hi)] = (xq[:, 0], xq[:, 1], True)

            def emit_wload(k):
                wt = wstream.tile([P, CC, 8, P], F16, name=f"wp_{k}", tag="wp")
                nc.sync.dma_start(out=wt, in_=wp_d[k, :, :, :, :])
                return wt

            # wp k=0 cc=0 leads the DMA queue so filter generation starts
            # ASAP; se/bias slot into its shadow, then the progressive
            # samples' activations stream behind wp k=1.
            # dummy first activation: hoists the one-time activation-table
            # load off the wgen-evacuation critical path
            scr = constp.tile([P, 1], F32)
            nc.vector.memset(scr, 0.0)
            scr2 = constp.tile([P, 1], F32)
            nc.scalar.activation(scr2, scr,
                                 mybir.ActivationFunctionType.Identity)

            wts = {}
            wt0 = wstream.tile([P, CC, 8, P], F16, name="wp_0", tag="wp")
            nc.sync.dma_start(out=wt0[:, 0], in_=wp_d[0, :, 0, :, :])
            se_sb = constp.tile([P, P], F16)
            nc.sync.dma_start(out=se_sb, in_=se_d[:, :])
            bias_sb = constp.tile([P, 1], F32)
            nc.sync.dma_start(out=bias_sb, in_=b_d[:, :])
            nc.sync.dma_start(out=wt0[:, 1], in_=wp_d[0, :, 1, :, :])
            wts[0] = wt0
            wts[1] = emit_wload(1)
            emit_xload_half(0, 0)
            wts[2] = emit_wload(2)
            emit_xload_half(1, 0)
            emit_xload_half(0, 1)
            wts[3] = emit_wload(3)
            emit_xload_half(1, 1)
            wts[4] = emit_wload(4)
            emit_xload_half(2, 0)
            wts[5] = emit_wload(5)
            emit_xload_half(2, 1)
            wts[6] = emit_wload(6)
            wts[7] = emit_wload(7)
            wts[8] = emit_wload(8)
            emit_xload_half(3, 0)
            emit_xload_half(3, 1)

            def emit_wgen(cc, k, sh, wt):
                # one sample-half: 8 matmuls of 64 cols -> 1 psum tile
                ps = wgps.tile([P, 8 * (SH * G)], F32)
                sse = se_sb[:, sh * SH * G:(sh + 1) * SH * G]
                for i in range(8):
                    nc.tensor.matmul(
                        ps[:, i * 64:(i + 1) * 64], wt[:, cc, i, :],
                        sse, start=True, stop=True,
                    )
                # pass 1 (Act): PSUM -> fp16 W slab, frees PSUM fast
                src = ps.rearrange("p (i s g) -> p i s g", i=8, s=SH, g=G)
                dst = w16[:, cc, k, sh * SH:(sh + 1) * SH, :].rearrange(
                    "p s (i g) -> p i s g", g=G)
                nc.scalar.activation(
                    dst, src, mybir.ActivationFunctionType.Identity)

            def emit_pass23(cc, k, sh):
                # pass 2 (DVE): w8 = fp8(W16) -- TensorCopy 2x all-SBUF mode.
                # pass 3: dwq = fp8(W16 - w8): DVE for cc0, Pool for cc1.
                wsrc = w16[:, cc, k, sh * SH:(sh + 1) * SH].rearrange(
                    "p s o -> p (s o)")
                wdst = w8[:, cc, k, sh * SH:(sh + 1) * SH].rearrange(
                    "p s o -> p (s o)")
                ddst = dwq[:, cc, k, sh * SH:(sh + 1) * SH].rearrange(
                    "p s o -> p (s o)")
                nc.vector.tensor_copy(out=wdst, in_=wsrc)
                eng = nc.vector if cc == 0 else nc.gpsimd
                eng.tensor_tensor(ddst, wsrc, wdst, mybir.AluOpType.subtract)

            def emit_conv_term(k, s, hi, pst, ti, first=False, last=False):
                ky, kx = k // KS, k % KS
                xv, dxv, half = xts[(s, hi)]
                st = (ky if half else hi * HO + ky) * FW + kx
                stat, mov = ((w8, xv), (w8, dxv), (dwq, xv))[ti]
                nc.tensor.matmul(
                    pst, stat[:, :, k, s, :], mov[:, :, st:st + NF],
                    start=first, stop=last,
                    perf_mode=mybir.MatmulPerfMode.DoubleRow,
                    skip_group_check=True,
                )

            outts = [None] * S

            def emit_evac(s, hi, pst):
                if hi == 0:
                    outts[s] = outp.tile([P, 2, HO, W], F16, name=f"ot_{s}",
                                         tag="ot")
                nc.scalar.activation(
                    outts[s][:, hi],
                    pst.rearrange("p (h w) -> p h w", w=FW)[:, :, 0:W],
                    mybir.ActivationFunctionType.Identity,
                    bias=bias_sb[:, 0:1], scale=1.0 / 16.0,
                )
                # per-half store so the final group's DMA tail is short
                nc.sync.dma_start(
                    out=out_d[s, :, hi * HO:(hi + 1) * HO, :],
                    in_=outts[s][:, hi],
                )

            _ctag = [0]

            def new_group():
                t = cvps.tile([P, NF], F32, name=f"cv{_ctag[0]}",
                              tag=f"cv_{_ctag[0] % 6}")
                _ctag[0] += 1
                return t

            # ---- phase A: wgen half 0 (samples 0-3) + progressive conv of
            # six staggered groups whose starts track their x-half DMAs.
            # LAGS[(s, hi)] = k-lag of the main/dx terms (dw is one more).
            LAGS = {(0, 0): 1, (1, 0): 2, (0, 1): 3,
                    (1, 1): 4, (2, 0): 5, (2, 1): 6}
            prog = {g: new_group() for g in LAGS}
            for k in range(NK):
                convs = []
                for (s, hi), lag in LAGS.items():
                    if k >= lag:
                        convs.append((k - lag, s, hi, 0, k == lag))
                        convs.append((k - lag, s, hi, 1, False))
                    if k >= lag + 1:
                        convs.append((k - lag - 1, s, hi, 2, False))

                def chunk(n):
                    for _ in range(n):
                        if convs:
                            ck, cs, chi, cti, cf = convs.pop(0)
                            emit_conv_term(ck, cs, chi, prog[(cs, chi)], cti,
                                           first=cf)

                emit_wgen(0, k, 0, wts[k])
                emit_wgen(1, k, 0, wts[k])
                emit_pass23(0, k, 0)
                emit_pass23(1, k, 0)
                if k < 4:
                    emit_xload(4 + k)
                chunk(len(convs))
            for (s, hi) in ((0, 0), (1, 0)):
                lag = LAGS[(s, hi)]
                pst = prog[(s, hi)]
                for kk in range(NK - lag, NK):
                    emit_conv_term(kk, s, hi, pst, 0)
                    emit_conv_term(kk, s, hi, pst, 1)
                for kk in range(NK - lag - 1, NK):
                    emit_conv_term(kk, s, hi, pst, 2,
                                   last=(kk == NK - 1))
                emit_evac(s, hi, pst)

            # ---- phase B: wgen half 1 (samples 4-7) interleaved with the
            # rest of the staggered groups plus the bursts of s3 (slab half
            # 0 is complete). Groups carry (tile, op-list, evac target).
            bqueue = []
            for (s, hi) in ((0, 1), (1, 1), (2, 0), (2, 1)):
                lag = LAGS[(s, hi)]
                ops = ([(kk, s, hi, ti) for kk in range(NK - lag, NK)
                        for ti in (0, 1)]
                       + [(kk, s, hi, 2) for kk in range(NK - lag - 1, NK)])
                bqueue.append((prog[(s, hi)], ops, s, hi))
            for hi in range(2):
                bqueue.append((None, [(k, 3, hi, ti) for k in range(NK)
                                      for ti in range(3)], 3, hi))

            bstate = {"cur": None, "ops": None, "s": 0, "hi": 0}

            def bchunk(n):
                while n > 0:
                    if bstate["cur"] is None:
                        if not bqueue:
                            return
                        tile, ops, s, hi = bqueue.pop(0)
                        bstate["cur"] = tile if tile is not None else new_group()
                        bstate["ops"] = list(ops)
                        bstate["s"], bstate["hi"] = s, hi
                        if tile is None:
                            ck, cs, chi, cti = bstate["ops"].pop(0)
                            emit_conv_term(ck, cs, chi, bstate["cur"], cti,
                                           first=True)
                    while n > 0 and bstate["ops"]:
                        ck, cs, chi, cti = bstate["ops"].pop(0)
                        last = not bstate["ops"]
                        emit_conv_term(ck, cs, chi, bstate["cur"], cti,
                                       last=last)
                        n -= 1
                    if not bstate["ops"]:
                        emit_evac(bstate["s"], bstate["hi"], bstate["cur"])
                        bstate["cur"] = None

            for k in range(NK):
                emit_wgen(0, k, 1, wts[k])
                bchunk(6)
                emit_wgen(1, k, 1, wts[k])
                emit_pass23(0, k, 1)
                emit_pass23(1, k, 1)
                bchunk(6)
            bchunk(10 ** 6)

            # ---- phase C: conv bursts of samples 4-7.
            for s in range(SH, S):
                for hi in range(2):
                    pst = new_group()
                    for k in range(NK):
                        for ti in range(3):
                            emit_conv_term(k, s, hi, pst, ti,
                                           first=(k == 0 and ti == 0),
                                           last=(k == NK - 1 and ti == 2))
                    emit_evac(s, hi, pst)

    nc.compile()
    return nc


def _get_nc():
    global _NC
    if _NC is None:
        _NC = _build_nc()
    return _NC


def _prep_core_inputs(inputs, inputs_se, weight, bias, bg, oh):
    # weight rows: r = o*(C*9) + c*9 + (ky*3+kx)  -> [O, C, 3, 3, NUM]
    wr = weight.reshape(O, C, KS, KS, NUM)
    wo = wr[oh * OC:(oh + 1) * OC].reshape(OC, C, NK, NUM)  # [o, c, k, n]
    # [j, g, cc, c_lo, k, n] -> [k, n, g, cc, j, c_lo]; p = n*16+g
    t = wo.reshape(8, G, CC, P, NK, NUM)
    wp = t.transpose(4, 5, 1, 2, 0, 3).reshape(NK, P, CC, 8, P)
    wp = np.ascontiguousarray(wp.astype(np.float16))

    # block-diagonal 16*se: [(n,g), (s,g')] nonzero iff g==g'
    se16 = (16.0 * inputs_se[bg * S:(bg + 1) * S]).astype(np.float32)  # [s, n]
    sebd = np.zeros((NUM, G, S, G), dtype=np.float32)
    for g in range(G):
        sebd[:, g, :, g] = se16.T
    sebd = sebd.reshape(P, P).astype(np.float16)

    # activations: fp8 split, width-29 row-flat layout with guard zeros
    x_core = inputs[bg * S:(bg + 1) * S].astype(np.float32)
    x8 = x_core.astype(E4NP)
    dx8 = (x_core - x8.astype(np.float32)).astype(E4NP)

    def to_flat(a):
        f = np.zeros((S, CC, P, NR, FW), dtype=E4NP)
        f[:, :, :, 1:H + 1, 1:W + 1] = a.reshape(S, CC, P, H, W)
        out = np.zeros((S, CC, P, FLAT), dtype=E4NP)
        out[:, :, :, :NR * FW] = f.reshape(S, CC, P, NR * FW)
        return out.transpose(0, 2, 1, 3)

    xq = np.ascontiguousarray(
        np.stack([to_flat(x8), to_flat(dx8)], axis=2))  # [S, P, 2, CC, FLAT]

    return {
        "xq": xq,
        "wp": wp,
        "sebd": sebd,
        "bias": np.ascontiguousarray(
            bias[oh * OC:(oh + 1) * OC].reshape(OC, 1), dtype=np.float32
        ),
    }


def kernel(inputs, inputs_se, weight, bias):
    inputs = np.asarray(inputs, dtype=np.float32)
    inputs_se = np.asarray(inputs_se, dtype=np.float32)
    weight = np.asarray(weight, dtype=np.float32)
    bias = np.asarray(bias, dtype=np.float32)

    nc = _get_nc()
    in_maps = []
    for core in range(NCORES):
        bg, oh = core // OHALF, core % OHALF
        in_maps.append(_prep_core_inputs(inputs, inputs_se, weight, bias, bg, oh))

    res = run_bass_kernel_spmd(nc, in_maps, list(range(NCORES))).results

    out = np.empty((B, O, H, W), dtype=np.float32)
    for core in range(NCORES):
        bg, oh = core // OHALF, core % OHALF
        out[bg * S:(bg + 1) * S, oh * OC:(oh + 1) * OC] = (
            res[core]["out"].astype(np.float32))
    return out


# revision 36
# speedup vs baseline: 1.0779x; 1.0667x over previous
"""Trainium2 Bass kernel for nn_DiverseRegDCConv2d.

Per-sample dynamic 3x3 conv: filters are generated per sample from an
8-column weight bank (wgen[b] = se[b] @ bank.T), then applied as a
standard 256->256 conv on 28x28 with padding 1.

Sharding (8 cores): 4 batch-groups x 2 out-channel halves. Each core
handles 8 samples x 128 out channels; the weight bank half it needs is
replicated across the 4 batch-groups. No cross-device communication.

Numerics: the conv runs on fp8e4 (e4m3) operands in DoubleRow perf
mode (K=256 per matmul, 0.5 cycles per output column) with a 3-term
residual split that recovers ~fp16 accuracy:

    out = (w8 (.) x8  +  w8 (.) dx8  +  dwq (.) x8) / 16

where the filters are generated on device at 16x scale (se pre-scaled
on host), w8 = fp8(W), dwq = fp8(W - w8), and the activations are
split on host as x8 = fp8(x), dx8 = fp8(x - x8). End-to-end relative
error vs the fp32 reference is ~1.5e-3.

Conv layout trick: activations are stored width-29 row-flattened
(left zero-pad column only; the right pad of row r aliases the left
zero of row r+1), so every 3x3 shifted window is a single contiguous
406-element run and the DoubleRow moving AP stays 3-D. Each PSUM row
has one discarded halo column (29 vs 28).

Filter generation stays on device and is SAMPLE-HALF blocked: the
block-diagonal 16*se operand's columns are sample-major, so a 64-col
slice generates filters for samples 0-3 (then 4-7) across all k.
Samples 2-3's conv bursts therefore overlap the second wgen half, and
samples 4-7's bursts run immediately after it — the PE never waits
long on filter generation. Evacuation is three passes so PSUM turns
around fast and the fp8 ops run SBUF->SBUF: (1) Act copy PSUM->W16,
(2) DVE quantize W16->w8 (TensorCopy in 2x all-SBUF mode),
(3) DVE/Pool subtract W16-w8 -> dwq.
"""

import sys

for _p in ("/opt/trn_rl_repo", "/root/.axon_site/_ro/trn_rl_repo"):
    if _p not in sys.path:
        sys.path.append(_p)

import ml_dtypes
import numpy as np

import concourse.bass as bass
import concourse.mybir as mybir
from concourse import bacc
from concourse.bass_utils import run_bass_kernel_spmd
from concourse.tile import TileContext

B, C, O, KS, H, W, NUM = 32, 256, 256, 3, 28, 28, 8
P = 128
NCORES = 8
BG, OHALF = 4, 2          # batch-groups x out-channel halves
S = B // BG               # samples per core = 8
OC = O // OHALF           # out channels per core = 128
CC = C // P               # input-channel chunks = 2
G = 16                    # (k,o)-blocks per wgen matmul column group
NK = KS * KS              # 9 kernel positions
SH = S // 2               # samples per wgen half = 4

FW = W + 1                # flat row width (left zero col, right pad aliased)
NR = H + 2                # padded rows
FLAT = NR * FW + 2        # + guard zeros for the bottom-right window overrun
HO = H // 2               # 14 output rows per psum group
NF = HO * FW              # 406 psum columns per group (1 halo col per row)
HFLAT = 512               # one output-half's input rows (466 used), padded
                          # to the 512B contiguous-chunk DMA threshold
FLATD = HO * FW + HFLAT   # dram row length so the hi=1 half-slice fits

NPROG = 2                 # samples whose conv groups run k-progressively

F32 = mybir.dt.float32
F16 = mybir.dt.float16
F8 = mybir.dt.float8e4
E4NP = ml_dtypes.float8_e4m3

_NC = None


def _build_nc():
    nc = bacc.Bacc()
    xq_d = nc.declare_dram_parameter("xq", [S, P, 2, CC, FLATD], F8, isOutput=False)
    wp_d = nc.declare_dram_parameter("wp", [NK, P, CC, 8, P], F16, isOutput=False)
    se_d = nc.declare_dram_parameter("sebd", [P, P], F16, isOutput=False)
    b_d = nc.declare_dram_parameter("bias", [P, 1], F32, isOutput=False)
    out_d = nc.declare_dram_parameter("out", [S, P, H, W], F16, isOutput=True)

    with TileContext(nc) as tc:
        with (
            tc.tile_pool(name="constp", bufs=1) as constp,
            tc.tile_pool(name="wstream", bufs=9) as wstream,
            tc.tile_pool(name="xpool", bufs=1) as xpool,
            tc.tile_pool(name="slabp", bufs=1) as slabp,
            tc.tile_pool(name="outp", bufs=4) as outp,
            tc.tile_pool(name="wgps", bufs=2, space="PSUM") as wgps,
            tc.tile_pool(name="cvps", bufs=1, space="PSUM") as cvps,
        ):
            # slabs: [c_lo, cc, k, s, oc]; conv lhsT = slab[:, :, k, s, :]
            w16 = slabp.tile([P, CC, NK, S, OC], F16)
            w8 = slabp.tile([P, CC, NK, S, OC], F8)
            dwq = slabp.tile([P, CC, NK, S, OC], F8)

            xts = {}   # (s, hi) -> (x8 view, dx8 view, half_layout)

            def emit_xload(s):
                # x8 and dx8 ride one DMA (fewer HWDGE setups); the conv
                # slices views of the combined tile
                xq = xpool.tile([P, 2, CC, FLAT], F8, name=f"xq_{s}",
                                tag=f"xq_{s}")
                nc.sync.dma_start(out=xq, in_=xq_d[s, :, :, :, :FLAT])
                for hi in range(2):
                    xts[(s, hi)] = (xq[:, 0], xq[:, 1], False)

            def emit_xload_half(s, hi):
                # one output-half's input rows only: finer DMA granularity so
                # early conv groups start sooner on the saturated front bus
                xq = xpool.tile([P, 2, CC, HFLAT], F8, name=f"xq_{s}_{hi}",
                                tag=f"xq_{s}_{hi}")
                off = hi * HO * FW
                nc.sync.dma_start(out=xq, in_=xq_d[s, :, :, :, off:off + HFLAT])
                xts[(s, hi)] = (xq[:, 0], xq[:, 1], True)

            def emit_wload(k):
                wt = wstream.tile([P, CC, 8, P], F16, name=f"wp_{k}", tag="wp")
                nc.sync.dma_start(out=wt, in_=wp_d[k, :, :, :, :])
                return wt

            # wp k=0 cc=0 leads the DMA queue so filter generation starts
            # ASAP; se/bias slot into its shadow, then the progressive
            # samples' activations stream behind wp k=1.
            # dummy first activation: hoists the one-time activation-table
            # load off the wgen-evacuation critical path
            scr = constp.tile([P, 1], F32)
            nc.vector.memset(scr, 0.0)
            scr2 = constp.tile([P, 1], F32)
            nc.scalar.activation(scr2, scr,
                                 mybir.ActivationFunctionType.Identity)

            wts = {}
            wt0 = wstream.tile([P, CC, 8, P], F16, name="wp_0", tag="wp")
            nc.sync.dma_start(out=wt0[:, 0], in_=wp_d[0, :, 0, :, :])
            se_sb = constp.tile([P, P], F16)
            nc.sync.dma_start(out=se_sb, in_=se_d[:, :])
            bias_sb = constp.tile([P, 1], F32)
            nc.sync.dma_start(out=bias_sb, in_=b_d[:, :])
            nc.sync.dma_start(out=wt0[:, 1], in_=wp_d[0, :, 1, :, :])
            wts[0] = wt0
            wts[1] = emit_wload(1)
            emit_xload_half(0, 0)
            wts[2] = emit_wload(2)
            emit_xload_half(1, 0)
            emit_xload_half(0, 1)
            wts[3] = emit_wload(3)
            emit_xload_half(1, 1)
            wts[4] = emit_wload(4)
            emit_xload_half(2, 0)
            wts[5] = emit_wload(5)
            emit_xload_half(2, 1)
            wts[6] = emit_wload(6)
            wts[7] = emit_wload(7)
            wts[8] = emit_wload(8)
            emit_xload_half(3, 0)
            emit_xload_half(3, 1)

            def emit_wgen(cc, k, sh, wt):
                # one sample-half: 8 matmuls of 64 cols -> 1 psum tile
                ps = wgps.tile([P, 8 * (SH * G)], F32)
                sse = se_sb[:, sh * SH * G:(sh + 1) * SH * G]
                for i in range(8):
                    nc.tensor.matmul(
                        ps[:, i * 64:(i + 1) * 64], wt[:, cc, i, :],
                        sse, start=True, stop=True,
                    )
                # pass 1 (Act): PSUM -> fp16 W slab, frees PSUM fast
                src = ps.rearrange("p (i s g) -> p i s g", i=8, s=SH, g=G)
                dst = w16[:, cc, k, sh * SH:(sh + 1) * SH, :].rearrange(
                    "p s (i g) -> p i s g", g=G)
                nc.scalar.activation(
                    dst, src, mybir.ActivationFunctionType.Identity)

            def emit_pass23(cc, k, sh):
                # pass 2 (DVE): w8 = fp8(W16) -- TensorCopy 2x all-SBUF mode.
                # pass 3: dwq = fp8(W16 - w8): DVE for cc0, Pool for cc1.
                wsrc = w16[:, cc, k, sh * SH:(sh + 1) * SH].rearrange(
                    "p s o -> p (s o)")
                wdst = w8[:, cc, k, sh * SH:(sh + 1) * SH].rearrange(
                    "p s o -> p (s o)")
                ddst = dwq[:, cc, k, sh * SH:(sh + 1) * SH].rearrange(
                    "p s o -> p (s o)")
                nc.vector.tensor_copy(out=wdst, in_=wsrc)
                eng = nc.vector if cc == 0 else nc.gpsimd
                eng.tensor_tensor(ddst, wsrc, wdst, mybir.AluOpType.subtract)

            def emit_conv_term(k, s, hi, pst, ti, first=False, last=False):
                ky, kx = k // KS, k % KS
                xv, dxv, half = xts[(s, hi)]
                st = (ky if half else hi * HO + ky) * FW + kx
                stat, mov = ((w8, xv), (w8, dxv), (dwq, xv))[ti]
                nc.tensor.matmul(
                    pst, stat[:, :, k, s, :], mov[:, :, st:st + NF],
                    start=first, stop=last,
                    perf_mode=mybir.MatmulPerfMode.DoubleRow,
                    skip_group_check=True,
                )

            outts = [None] * S

            def emit_evac(s, hi, pst):
                if hi == 0:
                    outts[s] = outp.tile([P, 2, HO, W], F16, name=f"ot_{s}",
                                         tag="ot")
                nc.scalar.activation(
                    outts[s][:, hi],
                    pst.rearrange("p (h w) -> p h w", w=FW)[:, :, 0:W],
                    mybir.ActivationFunctionType.Identity,
                    bias=bias_sb[:, 0:1], scale=1.0 / 16.0,
                )
                # per-half store so the final group's DMA tail is short
                nc.sync.dma_start(
                    out=out_d[s, :, hi * HO:(hi + 1) * HO, :],
                    in_=outts[s][:, hi],
                )

            _ctag = [0]

            def new_group():
                t = cvps.tile([P, NF], F32, name=f"cv{_ctag[0]}",
                              tag=f"cv_{_ctag[0] % 6}")
                _ctag[0] += 1
                return t

            # ---- phase A: wgen half 0 (samples 0-3) + progressive conv of
            # six staggered groups whose starts track their x-half DMAs.
            # LAGS[(s, hi)] = k-lag of the main/dx terms (dw is one more).
            LAGS = {(0, 0): 1, (1, 0): 2, (0, 1): 3,
                    (1, 1): 4, (2, 0): 5, (2, 1): 6}
            prog = {g: new_group() for g in LAGS}
            for k in range(NK):
                convs = []
                for (s, hi), lag in LAGS.items():
                    if k >= lag:
                        convs.append((k - lag, s, hi, 0, k == lag))
                        convs.append((k - lag, s, hi, 1, False))
                    if k >= lag + 1:
                        convs.append((k - lag - 1, s, hi, 2, False))

                def chunk(n):
                    for _ in range(n):
                        if convs:
                            ck, cs, chi, cti, cf = convs.pop(0)
                            emit_conv_term(ck, cs, chi, prog[(cs, chi)], cti,
                                           first=cf)

                emit_wgen(0, k, 0, wts[k])
                emit_wgen(1, k, 0, wts[k])
                emit_pass23(0, k, 0)
                emit_pass23(1, k, 0)
                if k < 4:
                    emit_xload(4 + k)
                chunk(len(convs))
            for (s, hi) in ((0, 0), (1, 0)):
                lag = LAGS[(s, hi)]
                pst = prog[(s, hi)]
                for kk in range(NK - lag, NK):
                    emit_conv_term(kk, s, hi, pst, 0)
                    emit_conv_term(kk, s, hi, pst, 1)
                for kk in range(NK - lag - 1, NK):
                    emit_conv_term(kk, s, hi, pst, 2,
                                   last=(kk == NK - 1))
                emit_evac(s, hi, pst)

            # ---- phase B: wgen half 1 (samples 4-7) interleaved with the
            # rest of the staggered groups plus the bursts of s3 (slab half
            # 0 is complete). Groups carry (tile, op-list, evac target).
            bqueue = []
            for (s, hi) in ((0, 1), (1, 1), (2, 0), (2, 1)):
                lag = LAGS[(s, hi)]
                ops = ([(kk, s, hi, ti) for kk in range(NK - lag, NK)
                        for ti in (0, 1)]
                       + [(kk, s, hi, 2) for kk in range(NK - lag - 1, NK)])
                bqueue.append((prog[(s, hi)], ops, s, hi))
            for hi in range(2):
                bqueue.append((None, [(k, 3, hi, ti) for k in range(NK)
                                      for ti in range(3)], 3, hi))

            bstate = {"cur": None, "ops": None, "s": 0, "hi": 0}

            def bchunk(n):
                while n > 0:
                    if bstate["cur"] is None:
                        if not bqueue:
                            return
                        tile, ops, s, hi = bqueue.pop(0)
                        bstate["cur"] = tile if tile is not None else new_group()
                        bstate["ops"] = list(ops)
                        bstate["s"], bstate["hi"] = s, hi
                        if tile is None:
                            ck, cs, chi, cti = bstate["ops"].pop(0)
                            emit_conv_term(ck, cs, chi, bstate["cur"], cti,
                                           first=True)
                    while n > 0 and bstate["ops"]:
                        ck, cs, chi, cti = bstate["ops"].pop(0)
                        last = not bstate["ops"]
                        emit_conv_term(ck, cs, chi, bstate["cur"], cti,
                                       last=last)
                        n -= 1
                    if not bstate["ops"]:
                        emit_evac(bstate["s"], bstate["hi"], bstate["cur"])
                        bstate["cur"] = None

            for k in range(NK):
                emit_wgen(0, k, 1, wts[k])
                bchunk(6)
                emit_wgen(1, k, 1, wts[k])
                emit_pass23(0, k, 1)
                emit_pass23(1, k, 1)
                bchunk(6)
            bchunk(10 ** 6)

            # ---- phase C: conv bursts of samples 4-7.
            for s in range(SH, S):
                for hi in range(2):
                    pst = new_group()
                    for k in range(NK):
                        for ti in range(3):
                            emit_conv_term(k, s, hi, pst, ti,
                                           first=(k == 0 and ti == 0),
                                           last=(k == NK - 1 and ti == 2))
                    emit_evac(s, hi, pst)

    nc.compile()
    return nc


def _get_nc():
    global _NC
    if _NC is None:
        _NC = _build_nc()
    return _NC


def _prep_core_inputs(inputs, inputs_se, weight, bias, bg, oh):
    # weight rows: r = o*(C*9) + c*9 + (ky*3+kx)  -> [O, C, 3, 3, NUM]
    wr = weight.reshape(O, C, KS, KS, NUM)
    wo = wr[oh * OC:(oh + 1) * OC].reshape(OC, C, NK, NUM)  # [o, c, k, n]
    # [j, g, cc, c_lo, k, n] -> [k, n, g, cc, j, c_lo]; p = n*16+g
    t = wo.reshape(8, G, CC, P, NK, NUM)
    wp = t.transpose(4, 5, 1, 2, 0, 3).reshape(NK, P, CC, 8, P)
    wp = np.ascontiguousarray(wp.astype(np.float16))

    # block-diagonal 16*se: [(n,g), (s,g')] nonzero iff g==g'
    se16 = (16.0 * inputs_se[bg * S:(bg + 1) * S]).astype(np.float32)  # [s, n]
    sebd = np.zeros((NUM, G, S, G), dtype=np.float32)
    for g in range(G):
        sebd[:, g, :, g] = se16.T
    sebd = sebd.reshape(P, P).astype(np.float16)

    # activations: fp8 split, width-29 row-flat layout with guard zeros
    x_core = inputs[bg * S:(bg + 1) * S].astype(np.float32)
    x8 = x_core.astype(E4NP)
    dx8 = (x_core - x8.astype(np.float32)).astype(E4NP)

    def to_flat(a):
        f = np.zeros((S, CC, P, NR, FW), dtype=E4NP)
        f[:, :, :, 1:H + 1, 1:W + 1] = a.reshape(S, CC, P, H, W)
        out = np.zeros((S, CC, P, FLATD), dtype=E4NP)
        out[:, :, :, :NR * FW] = f.reshape(S, CC, P, NR * FW)
        return out.transpose(0, 2, 1, 3)

    xq = np.ascontiguousarray(
        np.stack([to_flat(x8), to_flat(dx8)], axis=2))  # [S, P, 2, CC, FLAT]

    return {
        "xq": xq,
        "wp": wp,
        "sebd": sebd,
        "bias": np.ascontiguousarray(
            bias[oh * OC:(oh + 1) * OC].reshape(OC, 1), dtype=np.float32
        ),
    }


def kernel(inputs, inputs_se, weight, bias):
    inputs = np.asarray(inputs, dtype=np.float32)
    inputs_se = np.asarray(inputs_se, dtype=np.float32)
    weight = np.asarray(weight, dtype=np.float32)
    bias = np.asarray(bias, dtype=np.float32)

    nc = _get_nc()
    in_maps = []
    for core in range(NCORES):
        bg, oh = core // OHALF, core % OHALF
        in_maps.append(_prep_core_inputs(inputs, inputs_se, weight, bias, bg, oh))

    res = run_bass_kernel_spmd(nc, in_maps, list(range(NCORES))).results

    out = np.empty((B, O, H, W), dtype=np.float32)
    for core in range(NCORES):
        bg, oh = core // OHALF, core % OHALF
        out[bg * S:(bg + 1) * S, oh * OC:(oh + 1) * OC] = (
            res[core]["out"].astype(np.float32))
    return out


# revision 37
# speedup vs baseline: 1.1163x; 1.0356x over previous
"""Trainium2 Bass kernel for nn_DiverseRegDCConv2d.

Per-sample dynamic 3x3 conv: filters are generated per sample from an
8-column weight bank (wgen[b] = se[b] @ bank.T), then applied as a
standard 256->256 conv on 28x28 with padding 1.

Sharding (8 cores): 4 batch-groups x 2 out-channel halves. Each core
handles 8 samples x 128 out channels; the weight bank half it needs is
replicated across the 4 batch-groups. No cross-device communication.

Numerics: the conv runs on fp8e4 (e4m3) operands in DoubleRow perf
mode (K=256 per matmul, 0.5 cycles per output column) with a 3-term
residual split that recovers ~fp16 accuracy:

    out = (w8 (.) x8  +  w8 (.) dx8  +  dwq (.) x8) / 16

where the filters are generated on device at 16x scale (se pre-scaled
on host), w8 = fp8(W), dwq = fp8(W - w8), and the activations are
split on host as x8 = fp8(x), dx8 = fp8(x - x8). End-to-end relative
error vs the fp32 reference is ~1.5e-3.

Conv layout trick: activations are stored width-29 row-flattened
(left zero-pad column only; the right pad of row r aliases the left
zero of row r+1), so every 3x3 shifted window is a single contiguous
406-element run and the DoubleRow moving AP stays 3-D. Each PSUM row
has one discarded halo column (29 vs 28).

Filter generation stays on device and is SAMPLE-HALF blocked: the
block-diagonal 16*se operand's columns are sample-major, so a 64-col
slice generates filters for samples 0-3 (then 4-7) across all k.
Samples 2-3's conv bursts therefore overlap the second wgen half, and
samples 4-7's bursts run immediately after it — the PE never waits
long on filter generation. Evacuation is three passes so PSUM turns
around fast and the fp8 ops run SBUF->SBUF: (1) Act copy PSUM->W16,
(2) DVE quantize W16->w8 (TensorCopy in 2x all-SBUF mode),
(3) DVE/Pool subtract W16-w8 -> dwq.
"""

import sys

for _p in ("/opt/trn_rl_repo", "/root/.axon_site/_ro/trn_rl_repo"):
    if _p not in sys.path:
        sys.path.append(_p)

import ml_dtypes
import numpy as np

import concourse.bass as bass
import concourse.mybir as mybir
from concourse import bacc
from concourse.bass_utils import run_bass_kernel_spmd
from concourse.tile import TileContext

B, C, O, KS, H, W, NUM = 32, 256, 256, 3, 28, 28, 8
P = 128
NCORES = 8
BG, OHALF = 4, 2          # batch-groups x out-channel halves
S = B // BG               # samples per core = 8
OC = O // OHALF           # out channels per core = 128
CC = C // P               # input-channel chunks = 2
G = 16                    # (k,o)-blocks per wgen matmul column group
NK = KS * KS              # 9 kernel positions
SH = S // 2               # samples per wgen half = 4

FW = W + 1                # flat row width (left zero col, right pad aliased)
NR = H + 2                # padded rows
FLAT = NR * FW + 2        # + guard zeros for the bottom-right window overrun
HO = H // 2               # 14 output rows per psum group
NF = HO * FW              # 406 psum columns per group (1 halo col per row)
HFLAT = 512               # one output-half's input rows (466 used), padded
                          # to the 512B contiguous-chunk DMA threshold
FLATD = HO * FW + HFLAT   # dram row length so the hi=1 half-slice fits

NPROG = 2                 # samples whose conv groups run k-progressively

F32 = mybir.dt.float32
F16 = mybir.dt.float16
F8 = mybir.dt.float8e4
E4NP = ml_dtypes.float8_e4m3

_NC = None


def _build_nc():
    nc = bacc.Bacc()
    xq_d = nc.declare_dram_parameter("xq", [S, P, 2, CC, FLATD], F8, isOutput=False)
    wp_d = nc.declare_dram_parameter("wp", [NK, P, CC, 8, P], F16, isOutput=False)
    se_d = nc.declare_dram_parameter("sebd", [P, P], F16, isOutput=False)
    b_d = nc.declare_dram_parameter("bias", [P, 1], F32, isOutput=False)
    out_d = nc.declare_dram_parameter("out", [S, P, H, W], F16, isOutput=True)

    with TileContext(nc) as tc:
        with (
            tc.tile_pool(name="constp", bufs=1) as constp,
            tc.tile_pool(name="wstream", bufs=9) as wstream,
            tc.tile_pool(name="xpool", bufs=1) as xpool,
            tc.tile_pool(name="slabp", bufs=1) as slabp,
            tc.tile_pool(name="outp", bufs=4) as outp,
            tc.tile_pool(name="wgps", bufs=3, space="PSUM") as wgps,
            tc.tile_pool(name="cvps", bufs=1, space="PSUM") as cvps,
        ):
            # slabs: [c_lo, cc, k, s, oc]; conv lhsT = slab[:, :, k, s, :]
            w16 = slabp.tile([P, CC, NK, S, OC], F16)
            w8 = slabp.tile([P, CC, NK, S, OC], F8)
            dwq = slabp.tile([P, CC, NK, S, OC], F8)

            xts = {}   # (s, hi) -> (x8 view, dx8 view, half_layout)

            def emit_xload(s):
                # x8 and dx8 ride one DMA (fewer HWDGE setups); the conv
                # slices views of the combined tile
                xq = xpool.tile([P, 2, CC, FLAT], F8, name=f"xq_{s}",
                                tag=f"xq_{s}")
                nc.sync.dma_start(out=xq, in_=xq_d[s, :, :, :, :FLAT])
                for hi in range(2):
                    xts[(s, hi)] = (xq[:, 0], xq[:, 1], False)

            def emit_xload_half(s, hi):
                # one output-half's input rows only: finer DMA granularity so
                # early conv groups start sooner on the saturated front bus
                xq = xpool.tile([P, 2, CC, HFLAT], F8, name=f"xq_{s}_{hi}",
                                tag=f"xq_{s}_{hi}")
                off = hi * HO * FW
                nc.sync.dma_start(out=xq, in_=xq_d[s, :, :, :, off:off + HFLAT])
                xts[(s, hi)] = (xq[:, 0], xq[:, 1], True)

            def emit_wload(k):
                wt = wstream.tile([P, CC, 8, P], F16, name=f"wp_{k}", tag="wp")
                nc.sync.dma_start(out=wt, in_=wp_d[k, :, :, :, :])
                return wt

            # wp k=0 cc=0 leads the DMA queue so filter generation starts
            # ASAP; se/bias slot into its shadow, then the progressive
            # samples' activations stream behind wp k=1.
            # dummy first activation: hoists the one-time activation-table
            # load off the wgen-evacuation critical path
            scr = constp.tile([P, 1], F32)
            nc.vector.memset(scr, 0.0)
            scr2 = constp.tile([P, 1], F32)
            nc.scalar.activation(scr2, scr,
                                 mybir.ActivationFunctionType.Identity)

            wts = {}
            wt0 = wstream.tile([P, CC, 8, P], F16, name="wp_0", tag="wp")
            nc.sync.dma_start(out=wt0[:, 0], in_=wp_d[0, :, 0, :, :])
            se_sb = constp.tile([P, P], F16)
            nc.sync.dma_start(out=se_sb, in_=se_d[:, :])
            nc.sync.dma_start(out=wt0[:, 1], in_=wp_d[0, :, 1, :, :])
            wts[0] = wt0
            wts[1] = emit_wload(1)
            emit_xload_half(0, 0)
            wts[2] = emit_wload(2)
            emit_xload_half(1, 0)
            emit_xload_half(0, 1)
            wts[3] = emit_wload(3)
            emit_xload_half(1, 1)
            wts[4] = emit_wload(4)
            emit_xload_half(2, 0)
            wts[5] = emit_wload(5)
            emit_xload_half(2, 1)
            wts[6] = emit_wload(6)
            wts[7] = emit_wload(7)
            wts[8] = emit_wload(8)
            bias_sb = constp.tile([P, 1], F32)
            nc.sync.dma_start(out=bias_sb, in_=b_d[:, :])
            emit_xload_half(3, 0)
            emit_xload_half(3, 1)

            def emit_wgen(cc, k, sh, wt):
                # one sample-half: 8 matmuls of 64 cols -> 1 psum tile
                ps = wgps.tile([P, 8 * (SH * G)], F32)
                sse = se_sb[:, sh * SH * G:(sh + 1) * SH * G]
                for i in range(8):
                    nc.tensor.matmul(
                        ps[:, i * 64:(i + 1) * 64], wt[:, cc, i, :],
                        sse, start=True, stop=True,
                    )
                # pass 1 (Act): PSUM -> fp16 W slab, frees PSUM fast
                src = ps.rearrange("p (i s g) -> p i s g", i=8, s=SH, g=G)
                dst = w16[:, cc, k, sh * SH:(sh + 1) * SH, :].rearrange(
                    "p s (i g) -> p i s g", g=G)
                nc.scalar.activation(
                    dst, src, mybir.ActivationFunctionType.Identity)

            def emit_pass23(cc, k, sh):
                # pass 2 (DVE): w8 = fp8(W16) -- TensorCopy 2x all-SBUF mode.
                # pass 3: dwq = fp8(W16 - w8): DVE for cc0, Pool for cc1.
                wsrc = w16[:, cc, k, sh * SH:(sh + 1) * SH].rearrange(
                    "p s o -> p (s o)")
                wdst = w8[:, cc, k, sh * SH:(sh + 1) * SH].rearrange(
                    "p s o -> p (s o)")
                ddst = dwq[:, cc, k, sh * SH:(sh + 1) * SH].rearrange(
                    "p s o -> p (s o)")
                nc.vector.tensor_copy(out=wdst, in_=wsrc)
                eng = nc.vector if cc == 0 else nc.gpsimd
                eng.tensor_tensor(ddst, wsrc, wdst, mybir.AluOpType.subtract)

            def emit_conv_term(k, s, hi, pst, ti, first=False, last=False):
                ky, kx = k // KS, k % KS
                xv, dxv, half = xts[(s, hi)]
                st = (ky if half else hi * HO + ky) * FW + kx
                stat, mov = ((w8, xv), (w8, dxv), (dwq, xv))[ti]
                nc.tensor.matmul(
                    pst, stat[:, :, k, s, :], mov[:, :, st:st + NF],
                    start=first, stop=last,
                    perf_mode=mybir.MatmulPerfMode.DoubleRow,
                    skip_group_check=True,
                )

            outts = [None] * S

            def emit_evac(s, hi, pst):
                if hi == 0:
                    outts[s] = outp.tile([P, 2, HO, W], F16, name=f"ot_{s}",
                                         tag="ot")
                nc.scalar.activation(
                    outts[s][:, hi],
                    pst.rearrange("p (h w) -> p h w", w=FW)[:, :, 0:W],
                    mybir.ActivationFunctionType.Identity,
                    bias=bias_sb[:, 0:1], scale=1.0 / 16.0,
                )
                # per-half store so the final group's DMA tail is short
                nc.sync.dma_start(
                    out=out_d[s, :, hi * HO:(hi + 1) * HO, :],
                    in_=outts[s][:, hi],
                )

            _ctag = [0]

            def new_group():
                t = cvps.tile([P, NF], F32, name=f"cv{_ctag[0]}",
                              tag=f"cv_{_ctag[0] % 5}")
                _ctag[0] += 1
                return t

            # ---- phase A: wgen half 0 (samples 0-3) + progressive conv of
            # six staggered groups whose starts track their x-half DMAs.
            # LAGS[(s, hi)] = k-lag of the main/dx terms (dw is one more).
            LAGS = {(0, 0): 1, (1, 0): 2, (0, 1): 3,
                    (1, 1): 4, (2, 0): 5, (2, 1): 6}
            prog = {g: new_group() for g in LAGS}
            for k in range(NK):
                convs = []
                for (s, hi), lag in LAGS.items():
                    if k >= lag:
                        convs.append((k - lag, s, hi, 0, k == lag))
                        convs.append((k - lag, s, hi, 1, False))
                    if k >= lag + 1:
                        convs.append((k - lag - 1, s, hi, 2, False))

                def chunk(n):
                    for _ in range(n):
                        if convs:
                            ck, cs, chi, cti, cf = convs.pop(0)
                            emit_conv_term(ck, cs, chi, prog[(cs, chi)], cti,
                                           first=cf)

                emit_wgen(0, k, 0, wts[k])
                emit_wgen(1, k, 0, wts[k])
                emit_pass23(0, k, 0)
                emit_pass23(1, k, 0)
                if k <= 2:
                    # fill the DMA-starved early steps with the second
                    # sample-half's filter generation
                    emit_wgen(0, k, 1, wts[k])
                    emit_wgen(1, k, 1, wts[k])
                    emit_pass23(0, k, 1)
                    emit_pass23(1, k, 1)
                if k < 4:
                    emit_xload(4 + k)
                chunk(len(convs))
            for (s, hi) in ((0, 0), (1, 0)):
                lag = LAGS[(s, hi)]
                pst = prog[(s, hi)]
                for kk in range(NK - lag, NK):
                    emit_conv_term(kk, s, hi, pst, 0)
                    emit_conv_term(kk, s, hi, pst, 1)
                for kk in range(NK - lag - 1, NK):
                    emit_conv_term(kk, s, hi, pst, 2,
                                   last=(kk == NK - 1))
                emit_evac(s, hi, pst)

            # ---- phase B: wgen half 1 (samples 4-7) interleaved with the
            # rest of the staggered groups plus the bursts of s3 (slab half
            # 0 is complete). Groups carry (tile, op-list, evac target).
            bqueue = []
            for (s, hi) in ((0, 1), (1, 1), (2, 0), (2, 1)):
                lag = LAGS[(s, hi)]
                ops = ([(kk, s, hi, ti) for kk in range(NK - lag, NK)
                        for ti in (0, 1)]
                       + [(kk, s, hi, 2) for kk in range(NK - lag - 1, NK)])
                bqueue.append((prog[(s, hi)], ops, s, hi))
            for hi in range(2):
                bqueue.append((None, [(k, 3, hi, ti) for k in range(NK)
                                      for ti in range(3)], 3, hi))

            bstate = {"cur": None, "ops": None, "s": 0, "hi": 0}

            def bchunk(n):
                while n > 0:
                    if bstate["cur"] is None:
                        if not bqueue:
                            return
                        tile, ops, s, hi = bqueue.pop(0)
                        bstate["cur"] = tile if tile is not None else new_group()
                        bstate["ops"] = list(ops)
                        bstate["s"], bstate["hi"] = s, hi
                        if tile is None:
                            ck, cs, chi, cti = bstate["ops"].pop(0)
                            emit_conv_term(ck, cs, chi, bstate["cur"], cti,
                                           first=True)
                    while n > 0 and bstate["ops"]:
                        ck, cs, chi, cti = bstate["ops"].pop(0)
                        last = not bstate["ops"]
                        emit_conv_term(ck, cs, chi, bstate["cur"], cti,
                                       last=last)
                        n -= 1
                    if not bstate["ops"]:
                        emit_evac(bstate["s"], bstate["hi"], bstate["cur"])
                        bstate["cur"] = None

            for k in range(3, NK):
                emit_wgen(0, k, 1, wts[k])
                bchunk(6)
                emit_wgen(1, k, 1, wts[k])
                emit_pass23(0, k, 1)
                emit_pass23(1, k, 1)
                bchunk(6)
            bchunk(10 ** 6)

            # ---- phase C: conv bursts of samples 4-7.
            for s in range(SH, S):
                for hi in range(2):
                    pst = new_group()
                    for k in range(NK):
                        for ti in range(3):
                            emit_conv_term(k, s, hi, pst, ti,
                                           first=(k == 0 and ti == 0),
                                           last=(k == NK - 1 and ti == 2))
                    emit_evac(s, hi, pst)

    nc.compile()
    return nc


def _get_nc():
    global _NC
    if _NC is None:
        _NC = _build_nc()
    return _NC


def _prep_core_inputs(inputs, inputs_se, weight, bias, bg, oh):
    # weight rows: r = o*(C*9) + c*9 + (ky*3+kx)  -> [O, C, 3, 3, NUM]
    wr = weight.reshape(O, C, KS, KS, NUM)
    wo = wr[oh * OC:(oh + 1) * OC].reshape(OC, C, NK, NUM)  # [o, c, k, n]
    # [j, g, cc, c_lo, k, n] -> [k, n, g, cc, j, c_lo]; p = n*16+g
    t = wo.reshape(8, G, CC, P, NK, NUM)
    wp = t.transpose(4, 5, 1, 2, 0, 3).reshape(NK, P, CC, 8, P)
    wp = np.ascontiguousarray(wp.astype(np.float16))

    # block-diagonal 16*se: [(n,g), (s,g')] nonzero iff g==g'
    se16 = (16.0 * inputs_se[bg * S:(bg + 1) * S]).astype(np.float32)  # [s, n]
    sebd = np.zeros((NUM, G, S, G), dtype=np.float32)
    for g in range(G):
        sebd[:, g, :, g] = se16.T
    sebd = sebd.reshape(P, P).astype(np.float16)

    # activations: fp8 split, width-29 row-flat layout with guard zeros
    x_core = inputs[bg * S:(bg + 1) * S].astype(np.float32)
    x8 = x_core.astype(E4NP)
    dx8 = (x_core - x8.astype(np.float32)).astype(E4NP)

    def to_flat(a):
        f = np.zeros((S, CC, P, NR, FW), dtype=E4NP)
        f[:, :, :, 1:H + 1, 1:W + 1] = a.reshape(S, CC, P, H, W)
        out = np.zeros((S, CC, P, FLATD), dtype=E4NP)
        out[:, :, :, :NR * FW] = f.reshape(S, CC, P, NR * FW)
        return out.transpose(0, 2, 1, 3)

    xq = np.ascontiguousarray(
        np.stack([to_flat(x8), to_flat(dx8)], axis=2))  # [S, P, 2, CC, FLAT]

    return {
        "xq": xq,
        "wp": wp,
        "sebd": sebd,
        "bias": np.ascontiguousarray(
            bias[oh * OC:(oh + 1) * OC].reshape(OC, 1), dtype=np.float32
        ),
    }


def kernel(inputs, inputs_se, weight, bias):
    inputs = np.asarray(inputs, dtype=np.float32)
    inputs_se = np.asarray(inputs_se, dtype=np.float32)
    weight = np.asarray(weight, dtype=np.float32)
    bias = np.asarray(bias, dtype=np.float32)

    nc = _get_nc()
    in_maps = []
    for core in range(NCORES):
        bg, oh = core // OHALF, core % OHALF
        in_maps.append(_prep_core_inputs(inputs, inputs_se, weight, bias, bg, oh))

    res = run_bass_kernel_spmd(nc, in_maps, list(range(NCORES))).results

    out = np.empty((B, O, H, W), dtype=np.float32)
    for core in range(NCORES):
        bg, oh = core // OHALF, core % OHALF
        out[bg * S:(bg + 1) * S, oh * OC:(oh + 1) * OC] = (
            res[core]["out"].astype(np.float32))
    return out
